# revision 58
# baseline (speedup 1.0000x reference)
"""Trainium2 Bass kernel for nn_Disentangling_7009386627770.

Per-sample computation (reference):
  h = x @ W_enc + b_enc                       [T, HDIM]
  e = h*h; per-row top-64 indices; relay: row t uses top-64 set of row
  src_t = last t'<=t with |theta-511|>256 (else 0)
  mask[t] = indicator(top64(e[src_t]))        [T, HDIM]
  out = (h * mask) @ W_dec + b_dec            [T, IDIM]

Sharding: pure data parallel, one batch sample per NeuronCore (B=8).

Kernel strategy per core:
  - GEMM1 in native fp32 (exact enough for top-k selection vs f32 ref).
  - mask row = (e >= v64) where v64 = 64th largest energy of the row,
    found exactly via per-chunk top-8 (vector.max) + match_replace residual
    + 8 elimination rounds over 136 candidates.
  - relay realized as a [128x128] permutation matmul per 128-row tile plus
    a K=1 outer product with the previous tile's last mask row (carry).
  - GEMM2 in bf16 with PE-transposed masked h.
"""

import os
import sys
import types

import numpy as np

B, T, IDIM, HDIM, CDIM = 8, 2048, 512, 2048, 64
N_CORES = 8
NEG = -1.0e30
NT = T // 128        # 16 row tiles
KC = IDIM // 128     # 4 contraction chunks for GEMM1
HC = HDIM // 128     # 16 chunks (selection, GEMM2 contraction)

_CACHE = {}


def _install_patches():
    """Environment fixes: (1) walrus in this container rejects SP
    instructions with >1 sync wait -> distribute Tile's tail-drain waits
    over single-wait nops; (2) antenv.axon_hooks is absent -> recreate the
    NTFF profile hook via ctypes (used only when tracing)."""
    from concourse import tile as tile_mod
    from concourse.vector_clock import ScopedClock, VectorClock

    if not getattr(tile_mod.TileContext, "_drain_patched", False):
        def _patched(self, tick_clock, wait_clock):
            nc = self.nc
            g = tick_clock.global_clock
            n = len(g)
            for p in range(n):
                t = g[p]
                if t <= 0:
                    continue
                vec = [0] * n
                vec[p] = t
                nop_inst = nc.sync.nop()
                wait_clock.add_sem_waits(
                    nop_inst.ins, ScopedClock({None: VectorClock(vec)})
                )
            drain_inst = nc.sync.drain()
            wait_clock.add_sem_waits(
                drain_inst.ins,
                ScopedClock({None: g}),
                ScopedClock({None: g.copy()}),
            )
            nc.all_engine_barrier()
            assert self.sems is not None
            popped = nc._tile_sem_poison_stack.pop()
            assert popped is self._sem_poison
            nc.clear_and_free_semaphores(list(self.sems.allocated().values()))
            nc.all_engine_barrier()

        tile_mod.TileContext._drain_and_barrier = _patched
        tile_mod.TileContext._drain_patched = True

    if "antenv.axon_hooks" not in sys.modules:
        try:
            from trn_agent_boot.trn_boot import _ntff_profile_via_ctypes

            hook = _ntff_profile_via_ctypes("/opt/axon/libaxon_pjrt.so")
        except Exception:
            hook = None
        mod = types.ModuleType("antenv.axon_hooks")
        mod.get_axon_ntff_profile_hook = lambda: hook
        mod.set_axon_ntff_profile_hook = lambda h: None
        sys.modules["antenv.axon_hooks"] = mod


def _split_multi_waits(nc, mybir, max_waits=1):
    """This container's walrus rejects instructions carrying more than one
    sync wait. Move excess waits onto same-engine NoOps inserted just before
    the offending instruction (sequential waits == joint wait)."""
    import copy

    template = None
    for fn in nc.m.functions:
        for blk in fn.blocks:
            for inst in blk.instructions:
                if isinstance(inst, mybir.InstNoOp):
                    template = inst
                    break
            if template:
                break
        if template:
            break
    assert template is not None, "no NoOp template found"

    n_split = 0
    uid = [0]
    by_engine = {}
    for fn in nc.m.functions:
        for blk in fn.blocks:
            insts = list(blk.instructions)
            out = []
            changed = False
            for inst in insts:
                si = inst.sync_info
                waits = list(si.on_wait) if si and si.on_wait else []
                if len(waits) > max_waits:
                    eng = inst.engine
                    by_engine[str(eng)] = by_engine.get(str(eng), 0) + 1
                    extra, keep = waits[:-max_waits], waits[-max_waits:]
                    for w in extra:
                        nop = copy.deepcopy(template)
                        uid[0] += 1
                        nop.name = f"WS-{uid[0]}"
                        nop.engine = eng
                        nop.sync_info.on_wait = [w]
                        nop.sync_info.on_update = []
                        out.append(nop)
                        n_split += 1
                    si.on_wait = keep
                    changed = True
                out.append(inst)
            if changed:
                blk.instructions = out
    if n_split:
        print(f"[kernel] split {n_split} excess sync waits; engines: {by_engine}")


def build_nc(use_bias=True):
    import concourse.bass as bass
    import concourse.tile as tile
    from concourse import mybir
    from contextlib import ExitStack

    dt = mybir.dt
    f32, bf16, i32 = dt.float32, dt.bfloat16, dt.int32
    AF = mybir.ActivationFunctionType
    ALU = mybir.AluOpType

    nc = bass.Bass("TRN2", target_bir_lowering=False, debug=False, num_devices=1)

    x_d = nc.dram_tensor("x", [IDIM, T], f32, kind="ExternalInput").ap()
    th_d = nc.dram_tensor("theta", [NT, 128], i32, kind="ExternalInput").ap()
    we_d = nc.dram_tensor("W_enc", [IDIM, HDIM], f32, kind="ExternalInput").ap()
    be_d = nc.dram_tensor("b_enc", [1, HDIM], f32, kind="ExternalInput").ap()
    wd_d = nc.dram_tensor("W_dec", [HDIM, IDIM], f32, kind="ExternalInput").ap()
    bd_d = nc.dram_tensor("b_dec", [1, IDIM], f32, kind="ExternalInput").ap()
    out_d = nc.dram_tensor("out", [T, IDIM], f32, kind="ExternalOutput").ap()
    mask_d = nc.dram_tensor("mask", [T, HDIM], f32, kind="ExternalOutput").ap()

    GRP = 4          # tiles per group (front pass, then back pass)

    with tile.TileContext(nc) as tc, ExitStack() as ctx:
        pconst = ctx.enter_context(tc.tile_pool(name="const", bufs=1))
        pstage = ctx.enter_context(tc.tile_pool(name="stage", bufs=2))
        px = ctx.enter_context(tc.tile_pool(name="px", bufs=2))

        # prefetch tile 0's x before the bulk weight DMAs hit the queues
        x0_t = px.tile([128, 512], f32, tag="x_t", bufs=2, name="x0_t")
        for c in range(KC):
            nc.sync.dma_start(x0_t[:, 128 * c:128 * (c + 1)],
                              x_d[128 * c:128 * (c + 1), 0:128])

        if use_bias:
            benc = pconst.tile([1, HDIM], f32, tag="benc")
            nc.sync.dma_start(benc[:], be_d[:])
            bdec = pconst.tile([1, IDIM], bf16, tag="bdec")
            bdec_fs = pstage.tile([128, IDIM], f32, tag="wstage", name="bdec_fs")
            nc.sync.dma_start(bdec_fs[0:1, :], bd_d[:])
            nc.vector.tensor_copy(bdec[:], bdec_fs[0:1, :])

        # first GEMM1 K-chunk's weights converted first so tile 0 starts early
        whi, wlo = [], []
        st0 = pstage.tile([128, HDIM], f32, tag="westage", name="st0")
        nc.sync.dma_start(st0[:], we_d[0:128, :])
        hi0 = pconst.tile([128, HDIM], bf16, tag="whi0", name="hi0")
        nc.vector.tensor_copy(hi0[:], st0[:])
        lo0 = pconst.tile([128, HDIM], bf16, tag="wlo0", name="lo0")
        nc.gpsimd.tensor_tensor(lo0[:], st0[:], hi0[:], ALU.subtract)
        whi.append(hi0)
        wlo.append(lo0)

        ones_f = pconst.tile([1, 128], f32, tag="onesf")
        nc.vector.memset(ones_f[:], 1.0)
        ones_b = pconst.tile([1, 128], bf16, tag="onesb")
        nc.vector.memset(ones_b[:], 1.0)

        # identity matrices for PE transpose
        io_free_f = pconst.tile([128, 128], f32, tag="iofreef")
        nc.gpsimd.iota(io_free_f[:], pattern=[[1, 128]], base=0,
                       channel_multiplier=0, allow_small_or_imprecise_dtypes=True)
        io_part_f = pconst.tile([128, 1], f32, tag="iopartf")
        nc.gpsimd.iota(io_part_f[:], pattern=[[0, 1]], base=0,
                       channel_multiplier=1, allow_small_or_imprecise_dtypes=True)
        ident_f = pconst.tile([128, 128], f32, tag="identf")
        nc.vector.tensor_scalar(ident_f[:], io_free_f[:], io_part_f[:, 0:1], None,
                                ALU.is_equal)
        ident_b = pconst.tile([128, 128], bf16, tag="identb")
        nc.vector.tensor_copy(ident_b[:], ident_f[:])

        # ---------------- theta -> src16 [16, 128] -----------------
        # src16[p, j] = index of the most recent "moved" timestep <= 128p+j
        # (0 if none): hierarchical cummax of t*move[t].
        src16 = pconst.tile([NT, 128], f32, tag="src16")
        with tc.tile_pool(name="theta_tmp", bufs=4) as prow:
            th_i = prow.tile([NT, 128], i32, tag="th_i")
            nc.sync.dma_start(th_i[:], th_d[:])
            th_f = prow.tile([NT, 128], f32, tag="t16")
            nc.vector.tensor_copy(th_f[:], th_i[:])
            dev = prow.tile([NT, 128], f32, tag="t16")
            nc.vector.tensor_scalar(dev[:], th_f[:], -511.0, None, ALU.add)
            dev2 = prow.tile([NT, 128], f32, tag="t16")
            nc.vector.tensor_tensor(dev2[:], dev[:], dev[:], ALU.mult)
            move = prow.tile([NT, 128], f32, tag="t16")
            nc.vector.tensor_scalar(move[:], dev2[:], 65536.0, None, ALU.is_gt)
            io16 = prow.tile([NT, 128], f32, tag="t16")
            nc.gpsimd.iota(io16[:], pattern=[[1, 128]], base=0,
                           channel_multiplier=128,
                           allow_small_or_imprecise_dtypes=True)
            tmove = prow.tile([NT, 128], f32, tag="t16")
            nc.vector.tensor_tensor(tmove[:], move[:], io16[:], ALU.mult)
            z16 = prow.tile([NT, 128], f32, tag="t16")
            nc.vector.memset(z16[:], 0.0)
            s1 = prow.tile([NT, 128], f32, tag="t16")
            nc.vector.tensor_tensor_scan(s1[:], tmove[:], z16[:], 0.0,
                                         ALU.max, ALU.add)
            # segment maxes live in s1[:, 127]; exclusive cross-segment
            # cummax via DVE 32x32 stream-transposes (keeps PE free for GEMM1)
            tpa = prow.tile([32, 32], f32, tag="tpa")
            nc.vector.memset(tpa[:], 0.0)
            nc.vector.tensor_copy(tpa[0:NT, 0:1], s1[:, 127:128])
            tpb = prow.tile([32, 32], f32, tag="tpb")
            nc.vector.transpose(tpb[:], tpa[:])
            incl = prow.tile([1, NT], f32, tag="incl")
            nc.vector.tensor_tensor_scan(incl[:], tpb[0:1, 0:NT],
                                         tpb[0:1, 0:NT], 0.0,
                                         ALU.max, ALU.bypass)
            nc.vector.memset(tpa[0:1, 0:1], 0.0)
            nc.vector.tensor_copy(tpa[0:1, 1:NT], incl[:, 0:NT - 1])
            nc.vector.transpose(tpb[:], tpa[:])
            nc.vector.tensor_scalar(src16[:], s1[:], tpb[0:NT, 0:1], None,
                                    ALU.max)
        src_g = pconst.tile([1, T], f32, tag="srcg")
        nc.gpsimd.dma_start(src_g[:], src16[:])

        # ---------------- weights ----------------
        # W_enc split into bf16 hi+lo for the 3-term split GEMM1
        for kc in range(1, KC):
            st = pstage.tile([128, HDIM], f32, tag="westage")
            nc.sync.dma_start(st[:], we_d[128 * kc:128 * (kc + 1), :])
            hi = pconst.tile([128, HDIM], bf16, tag=f"whi{kc}")
            nc.vector.tensor_copy(hi[:], st[:])
            lo = pconst.tile([128, HDIM], bf16, tag=f"wlo{kc}")
            nc.gpsimd.tensor_tensor(lo[:], st[:], hi[:], ALU.subtract)
            whi.append(hi)
            wlo.append(lo)
        wdec_all = pconst.tile([128, HC * IDIM], bf16, tag="wdec")
        for hc in range(HC):
            st = pstage.tile([128, IDIM], f32, tag="wstage")
            nc.sync.dma_start(st[:], wd_d[128 * hc:128 * (hc + 1), :])
            nc.vector.tensor_copy(wdec_all[:, IDIM * hc:IDIM * (hc + 1)], st[:])
        wdec = [wdec_all[:, IDIM * hc:IDIM * (hc + 1)] for hc in range(HC)]

        carry = []
        for j in range(2):
            cro = pconst.tile([1, HDIM], bf16, tag=f"carry{j}", name=f"carry{j}")
            carry.append(cro)

        pbig = ctx.enter_context(tc.tile_pool(name="pbig", bufs=2))
        pew = ctx.enter_context(tc.tile_pool(name="pew", bufs=1))
        pmed = ctx.enter_context(tc.tile_pool(name="pmed", bufs=2))
        psm = ctx.enter_context(tc.tile_pool(name="psm", bufs=2))
        pph = ctx.enter_context(tc.tile_pool(name="pph", bufs=2, space="PSUM"))
        pps = ctx.enter_context(tc.tile_pool(name="pps", bufs=2, space="PSUM"))

        h_bfs, mpress, scrs = {}, {}, {}

        def gemm1_half(i, half, xhi, xlo, h_bf, e_sq, cand, e_work):
            ph = pph.tile([128, 1024], f32, tag="h1024", bufs=2, name="ph")
            # 3-term split: xhi@whi + xhi@wlo + xlo@whi, lhsT-major for LDW reuse
            for kc in range(KC):
                lx = xhi[:, 128 * kc:128 * (kc + 1)]
                for sx in range(2):
                    ns = 1024 * half + 512 * sx
                    nc.tensor.matmul(ph[:, 512 * sx:512 * (sx + 1)],
                                     lhsT=lx, rhs=whi[kc][:, ns:ns + 512],
                                     start=(kc == 0), stop=False)
                for sx in range(2):
                    ns = 1024 * half + 512 * sx
                    nc.tensor.matmul(ph[:, 512 * sx:512 * (sx + 1)],
                                     lhsT=lx, rhs=wlo[kc][:, ns:ns + 512],
                                     start=False, stop=False)
            for kc in range(KC):
                lx = xlo[:, 128 * kc:128 * (kc + 1)]
                for sx in range(2):
                    ns = 1024 * half + 512 * sx
                    nc.tensor.matmul(
                        ph[:, 512 * sx:512 * (sx + 1)],
                        lhsT=lx, rhs=whi[kc][:, ns:ns + 512],
                        start=False,
                        stop=(not use_bias and kc == KC - 1))
            if use_bias:
                for sx in range(2):
                    ns = 1024 * half + 512 * sx
                    nc.tensor.matmul(ph[:, 512 * sx:512 * (sx + 1)],
                                     lhsT=ones_f[:],
                                     rhs=benc[:, ns:ns + 512],
                                     start=False, stop=True)
            nc.scalar.activation(e_sq[:, 1024 * half:1024 * (half + 1)],
                                 ph[:], AF.Square)
            nc.scalar.activation(h_bf[:, 1024 * half:1024 * (half + 1)],
                                 ph[:], AF.Copy)
            for c in range(8 * half, 8 * (half + 1)):
                nc.vector.max(cand[:, 8 * c:8 * (c + 1)],
                              e_sq[:, 128 * c:128 * (c + 1)])
            for c in range(8 * half, 8 * (half + 1)):
                nc.vector.match_replace(e_work[:, 128 * c:128 * (c + 1)],
                                        cand[:, 8 * c:8 * (c + 1)],
                                        e_sq[:, 128 * c:128 * (c + 1)], NEG)

        def front_a(i):
            base = 128 * i
            # x arrives pre-transposed [IDIM, T]: DMA the 4 K-chunks of this
            # tile's columns, split hi/lo on GPSIMD
            if i == 0:
                x_t = x0_t
            else:
                x_t = px.tile([128, 512], f32, tag="x_t", bufs=2, name="x_t")
                for c in range(KC):
                    nc.sync.dma_start(x_t[:, 128 * c:128 * (c + 1)],
                                      x_d[128 * c:128 * (c + 1),
                                          base:base + 128])
            xhi = px.tile([128, 512], bf16, tag="xhi", name="xhi")
            nc.gpsimd.tensor_copy(xhi[:], x_t[:])
            xlo = px.tile([128, 512], bf16, tag="xlo", name="xlo")
            nc.gpsimd.tensor_tensor(xlo[:], x_t[:], xhi[:], ALU.subtract)

            scr = psm.tile([128, 1024], f32, tag="scr", name="scr")
            cand = scr[:, 136:272]
            h_bf = pmed.tile([128, HDIM], bf16, tag="hbf", bufs=3,
                             name="h_bf")
            e_sq = pbig.tile([128, HDIM], f32, tag="e",
                             bufs=3 if not use_bias else 2, name="e_sq")
            h_bfs[i] = h_bf
            e_work = pew.tile([128, HDIM], f32, tag="ew", name="e_work")
            scrs[i] = (scr, xhi, xlo, e_sq, e_work)
            gemm1_half(i, 0, xhi, xlo, h_bf, e_sq, cand, e_work)

        def front_b(i):
            scr, xhi, xlo, e_sq, e_work = scrs[i]
            cand = scr[:, 136:272]
            h_bf = h_bfs[i]
            gemm1_half(i, 1, xhi, xlo, h_bf, e_sq, cand, e_work)

            # exact 64th-largest per row
            nc.vector.max(cand[:, 128:136], e_work[:])
            cur = cand
            c2 = [scr[:, 288:424], scr[:, 432:568]]
            t8s = [scr[:, 576:584], scr[:, 592:600]]
            for r in range(8):
                t8 = t8s[r % 2]
                nc.vector.max(t8, cur[:, 0:136])
                if r < 7:
                    nxt = c2[r % 2]
                    nc.vector.match_replace(nxt, t8, cur[:, 0:136], NEG)
                    cur = nxt
            v64 = scr[:, 608:609]
            nc.vector.tensor_copy(v64, t8[:, 7:8])

            mpres = pmed.tile([128, HDIM], bf16, tag="mp", bufs=3,
                              name="mpres")
            nc.vector.tensor_scalar(mpres[:], e_sq[:], v64[:, 0:1], None,
                                    ALU.is_ge)
            mpress[i] = mpres
            del scrs[i]

        hms = {}

        def back_a(i):
            base = 128 * i
            h_bf = h_bfs.pop(i)
            mpres = mpress.pop(i)
            # permutation matrix for the relay: broadcast src row, compare
            scb = psm.tile([128, 256], bf16, tag="scb", name="scb")
            shifted = psm.tile([128, 8], f32, tag="shifted", name="shifted")
            P_T = scb[:, 0:128]
            cind = scb[0:1, 128:256]
            psb = pps.tile([128, 128], f32, tag="back", bufs=4, name="psb")
            nc.tensor.matmul(psb[:], lhsT=ones_f[:],
                             rhs=src_g[0:1, base:base + 128],
                             start=True, stop=True)
            nc.vector.tensor_scalar(shifted[:, 0:1], io_part_f[:], float(base),
                                    None, ALU.add)
            nc.vector.tensor_scalar(P_T, psb[:], shifted[:, 0:1], None,
                                    ALU.is_equal)
            if i > 0:
                nc.vector.tensor_scalar(cind, src_g[0:1, base:base + 128],
                                        float(base), None, ALU.is_lt)

            mask_sb = pbig.tile([128, HDIM], f32, tag="msk", name="mask_sb")
            hm = pmed.tile([128, HDIM], bf16, tag="hm", name="hm")
            for ns in range(4):
                pm = pps.tile([128, 512], f32, tag="back", bufs=4, name="pm")
                sl = slice(512 * ns, 512 * (ns + 1))
                nc.tensor.matmul(pm[:], lhsT=P_T, rhs=mpres[:, sl],
                                 start=True, stop=(i == 0))
                if i > 0:
                    nc.tensor.matmul(pm[:], lhsT=cind,
                                     rhs=carry[(i - 1) % 2][:, sl],
                                     start=False, stop=True)
                nc.vector.tensor_copy(mask_sb[:, sl], pm[:])
                nc.gpsimd.tensor_tensor(hm[:, sl], h_bf[:, sl], mask_sb[:, sl],
                                        ALU.mult)
            nc.sync.dma_start(mask_d[base:base + 128, :], mask_sb[:])
            if i < NT - 1:
                nc.gpsimd.dma_start(carry[i % 2][:], mask_sb[127:128, :])
            hms[i] = hm

        def back_b(i):
            base = 128 * i
            hm = hms.pop(i)
            hmT = pmed.tile([128, HDIM], bf16, tag="hmT", name="hmT")
            po = pps.tile([128, 512], f32, tag="back", bufs=4, name="po")
            for g in range(4):
                pt = pps.tile([128, 512], bf16, tag="back", bufs=4, name="pt")
                for j in range(4):
                    hc = 4 * g + j
                    nc.tensor.transpose(pt[:, 128 * j:128 * (j + 1)],
                                        hm[:, 128 * hc:128 * (hc + 1)],
                                        ident_b[:])
                nc.scalar.activation(hmT[:, 512 * g:512 * (g + 1)], pt[:],
                                     AF.Copy)
                for j in range(4):
                    hc = 4 * g + j
                    nc.tensor.matmul(po[:],
                                     lhsT=hmT[:, 128 * hc:128 * (hc + 1)],
                                     rhs=wdec[hc][:], start=(hc == 0),
                                     stop=(not use_bias and hc == HC - 1))
            if use_bias:
                nc.tensor.matmul(po[:], lhsT=ones_b[:], rhs=bdec[:],
                                 start=False, stop=True)
            out_sb = px.tile([128, IDIM], f32, tag="out_sb", name="out_sb")
            nc.scalar.activation(out_sb[:], po[:], AF.Copy)
            nc.sync.dma_start(out_d[base:base + 128, :], out_sb[:])

        for i in range(NT + 1):
            if i < NT:
                front_a(i)
                front_b(i)
            if i >= 1:
                back_a(i - 1)
                back_b(i - 1)

    _split_multi_waits(nc, mybir)
    return nc


def run_on_hw(inputs, trace=False, trace_kwargs=None):
    _install_patches()
    from concourse.bass_utils import run_bass_kernel_spmd

    use_bias = bool(np.any(inputs["b_enc"]) or np.any(inputs["b_dec"]))
    key = f"nc_bias{int(use_bias)}"
    if key not in _CACHE:
        _CACHE[key] = build_nc(use_bias=use_bias)
    nc = _CACHE[key]

    x = np.ascontiguousarray(inputs["x"], dtype=np.float32)
    theta = np.ascontiguousarray(inputs["theta"], dtype=np.int32)
    in_maps = []
    for i in range(N_CORES):
        in_maps.append({
            "x": np.ascontiguousarray(x[i].T),
            "theta": theta[i].reshape(NT, 128),
            "W_enc": np.ascontiguousarray(inputs["W_enc"], dtype=np.float32),
            "b_enc": np.ascontiguousarray(inputs["b_enc"], dtype=np.float32)[None, :],
            "W_dec": np.ascontiguousarray(inputs["W_dec"], dtype=np.float32),
            "b_dec": np.ascontiguousarray(inputs["b_dec"], dtype=np.float32)[None, :],
        })
    try:
        res = run_bass_kernel_spmd(nc, in_maps, core_ids=list(range(N_CORES)),
                                   trace=trace, **(trace_kwargs or {}))
    except Exception:
        # transient device/runtime hiccups (e.g. NRT_EXEC_UNIT_UNRECOVERABLE)
        # have been observed to succeed on retry with the same NEFF
        res = run_bass_kernel_spmd(nc, in_maps, core_ids=list(range(N_CORES)),
                                   trace=False)
    out = np.stack([res.results[i]["out"] for i in range(N_CORES)])
    mask = np.stack([res.results[i]["mask"] for i in range(N_CORES)])
    return (out, mask), res


def kernel(x, theta, W_enc, b_enc, W_dec, b_dec):
    (out, mask), _ = run_on_hw(dict(x=x, theta=theta, W_enc=W_enc, b_enc=b_enc,
                                    W_dec=W_dec, b_dec=b_dec))
    return out, mask


if __name__ == "__main__":
    rng = np.random.default_rng(0)
    ins = {
        "x": rng.standard_normal((B, T, IDIM)).astype(np.float32),
        "theta": rng.integers(0, 1024, size=(B, T)).astype(np.int32),
        "W_enc": (rng.standard_normal((IDIM, HDIM)) * 0.02).astype(np.float32),
        "b_enc": np.zeros((HDIM,), np.float32),
        "W_dec": (rng.standard_normal((HDIM, IDIM)) * 0.02).astype(np.float32),
        "b_dec": np.zeros((IDIM,), np.float32),
    }
    (out, mask), res = run_on_hw(ins)
    print("out", out.shape, "mask", mask.shape, "ones/row",
          mask.sum(-1).mean())


# revision 59
# speedup vs baseline: 1.0569x; 1.0569x over previous
"""Trainium2 Bass kernel for nn_Disentangling_7009386627770.

Per-sample computation (reference):
  h = x @ W_enc + b_enc                       [T, HDIM]
  e = h*h; per-row top-64 indices; relay: row t uses top-64 set of row
  src_t = last t'<=t with |theta-511|>256 (else 0)
  mask[t] = indicator(top64(e[src_t]))        [T, HDIM]
  out = (h * mask) @ W_dec + b_dec            [T, IDIM]

Sharding: pure data parallel, one batch sample per NeuronCore (B=8).

Kernel strategy per core:
  - GEMM1 in native fp32 (exact enough for top-k selection vs f32 ref).
  - mask row = (e >= v64) where v64 = 64th largest energy of the row,
    found exactly via per-chunk top-8 (vector.max) + match_replace residual
    + 8 elimination rounds over 136 candidates.
  - relay realized as a [128x128] permutation matmul per 128-row tile plus
    a K=1 outer product with the previous tile's last mask row (carry).
  - GEMM2 in bf16 with PE-transposed masked h.
"""

import os
import sys
import types

import numpy as np

B, T, IDIM, HDIM, CDIM = 8, 2048, 512, 2048, 64
N_CORES = 8
NEG = -1.0e30
NT = T // 128        # 16 row tiles
KC = IDIM // 128     # 4 contraction chunks for GEMM1
HC = HDIM // 128     # 16 chunks (selection, GEMM2 contraction)

_CACHE = {}


def _install_patches():
    """Environment fixes: (1) walrus in this container rejects SP
    instructions with >1 sync wait -> distribute Tile's tail-drain waits
    over single-wait nops; (2) antenv.axon_hooks is absent -> recreate the
    NTFF profile hook via ctypes (used only when tracing)."""
    from concourse import tile as tile_mod
    from concourse.vector_clock import ScopedClock, VectorClock

    if not getattr(tile_mod.TileContext, "_drain_patched", False):
        def _patched(self, tick_clock, wait_clock):
            nc = self.nc
            g = tick_clock.global_clock
            n = len(g)
            for p in range(n):
                t = g[p]
                if t <= 0:
                    continue
                vec = [0] * n
                vec[p] = t
                nop_inst = nc.sync.nop()
                wait_clock.add_sem_waits(
                    nop_inst.ins, ScopedClock({None: VectorClock(vec)})
                )
            drain_inst = nc.sync.drain()
            wait_clock.add_sem_waits(
                drain_inst.ins,
                ScopedClock({None: g}),
                ScopedClock({None: g.copy()}),
            )
            nc.all_engine_barrier()
            assert self.sems is not None
            popped = nc._tile_sem_poison_stack.pop()
            assert popped is self._sem_poison
            nc.clear_and_free_semaphores(list(self.sems.allocated().values()))
            nc.all_engine_barrier()

        tile_mod.TileContext._drain_and_barrier = _patched
        tile_mod.TileContext._drain_patched = True

    if "antenv.axon_hooks" not in sys.modules:
        try:
            from trn_agent_boot.trn_boot import _ntff_profile_via_ctypes

            hook = _ntff_profile_via_ctypes("/opt/axon/libaxon_pjrt.so")
        except Exception:
            hook = None
        mod = types.ModuleType("antenv.axon_hooks")
        mod.get_axon_ntff_profile_hook = lambda: hook
        mod.set_axon_ntff_profile_hook = lambda h: None
        sys.modules["antenv.axon_hooks"] = mod


def _split_multi_waits(nc, mybir, max_waits=1):
    """This container's walrus rejects instructions carrying more than one
    sync wait. Move excess waits onto same-engine NoOps inserted just before
    the offending instruction (sequential waits == joint wait)."""
    import copy

    template = None
    for fn in nc.m.functions:
        for blk in fn.blocks:
            for inst in blk.instructions:
                if isinstance(inst, mybir.InstNoOp):
                    template = inst
                    break
            if template:
                break
        if template:
            break
    assert template is not None, "no NoOp template found"

    n_split = 0
    uid = [0]
    by_engine = {}
    for fn in nc.m.functions:
        for blk in fn.blocks:
            insts = list(blk.instructions)
            out = []
            changed = False
            for inst in insts:
                si = inst.sync_info
                waits = list(si.on_wait) if si and si.on_wait else []
                if len(waits) > max_waits:
                    eng = inst.engine
                    by_engine[str(eng)] = by_engine.get(str(eng), 0) + 1
                    extra, keep = waits[:-max_waits], waits[-max_waits:]
                    for w in extra:
                        nop = copy.deepcopy(template)
                        uid[0] += 1
                        nop.name = f"WS-{uid[0]}"
                        nop.engine = eng
                        nop.sync_info.on_wait = [w]
                        nop.sync_info.on_update = []
                        out.append(nop)
                        n_split += 1
                    si.on_wait = keep
                    changed = True
                out.append(inst)
            if changed:
                blk.instructions = out
    if n_split:
        print(f"[kernel] split {n_split} excess sync waits; engines: {by_engine}")


def build_nc(use_bias=True):
    import concourse.bass as bass
    import concourse.tile as tile
    from concourse import mybir
    from contextlib import ExitStack

    dt = mybir.dt
    f32, bf16, i32 = dt.float32, dt.bfloat16, dt.int32
    AF = mybir.ActivationFunctionType
    ALU = mybir.AluOpType

    nc = bass.Bass("TRN2", target_bir_lowering=False, debug=False, num_devices=1)

    x_d = nc.dram_tensor("x", [IDIM, T], f32, kind="ExternalInput").ap()
    th_d = nc.dram_tensor("theta", [NT, 128], i32, kind="ExternalInput").ap()
    we_d = nc.dram_tensor("W_enc", [IDIM, HDIM], f32, kind="ExternalInput").ap()
    be_d = nc.dram_tensor("b_enc", [1, HDIM], f32, kind="ExternalInput").ap()
    wd_d = nc.dram_tensor("W_dec", [HDIM, IDIM], f32, kind="ExternalInput").ap()
    bd_d = nc.dram_tensor("b_dec", [1, IDIM], f32, kind="ExternalInput").ap()
    out_d = nc.dram_tensor("out", [T, IDIM], f32, kind="ExternalOutput").ap()
    mask_d = nc.dram_tensor("mask", [T, HDIM], f32, kind="ExternalOutput").ap()

    GRP = 4          # tiles per group (front pass, then back pass)

    with tile.TileContext(nc) as tc, ExitStack() as ctx:
        pconst = ctx.enter_context(tc.tile_pool(name="const", bufs=1))
        pstage = ctx.enter_context(tc.tile_pool(name="stage", bufs=2))
        px = ctx.enter_context(tc.tile_pool(name="px", bufs=2))

        # prefetch tile 0's x before the bulk weight DMAs hit the queues
        x0_t = px.tile([128, 512], f32, tag="x_t", bufs=2, name="x0_t")
        for c in range(KC):
            nc.sync.dma_start(x0_t[:, 128 * c:128 * (c + 1)],
                              x_d[128 * c:128 * (c + 1), 0:128])

        if use_bias:
            benc = pconst.tile([1, HDIM], f32, tag="benc")
            nc.sync.dma_start(benc[:], be_d[:])
            bdec = pconst.tile([1, IDIM], bf16, tag="bdec")
            bdec_fs = pstage.tile([128, IDIM], f32, tag="wstage", name="bdec_fs")
            nc.sync.dma_start(bdec_fs[0:1, :], bd_d[:])
            nc.vector.tensor_copy(bdec[:], bdec_fs[0:1, :])

        # first GEMM1 K-chunk's weights converted first so tile 0 starts early
        whi, wlo = [], []
        st0 = pstage.tile([128, HDIM], f32, tag="westage", name="st0")
        nc.sync.dma_start(st0[:], we_d[0:128, :])
        hi0 = pconst.tile([128, HDIM], bf16, tag="whi0", name="hi0")
        nc.vector.tensor_copy(hi0[:], st0[:])
        lo0 = pconst.tile([128, HDIM], bf16, tag="wlo0", name="lo0")
        nc.gpsimd.tensor_tensor(lo0[:], st0[:], hi0[:], ALU.subtract)
        whi.append(hi0)
        wlo.append(lo0)

        ones_f = pconst.tile([1, 128], f32, tag="onesf")
        nc.vector.memset(ones_f[:], 1.0)
        ones_b = pconst.tile([1, 128], bf16, tag="onesb")
        nc.vector.memset(ones_b[:], 1.0)

        # identity matrices for PE transpose
        io_free_f = pconst.tile([128, 128], f32, tag="iofreef")
        nc.gpsimd.iota(io_free_f[:], pattern=[[1, 128]], base=0,
                       channel_multiplier=0, allow_small_or_imprecise_dtypes=True)
        io_part_f = pconst.tile([128, 1], f32, tag="iopartf")
        nc.gpsimd.iota(io_part_f[:], pattern=[[0, 1]], base=0,
                       channel_multiplier=1, allow_small_or_imprecise_dtypes=True)
        ident_f = pconst.tile([128, 128], f32, tag="identf")
        nc.vector.tensor_scalar(ident_f[:], io_free_f[:], io_part_f[:, 0:1], None,
                                ALU.is_equal)
        ident_b = pconst.tile([128, 128], bf16, tag="identb")
        nc.vector.tensor_copy(ident_b[:], ident_f[:])

        # ---------------- theta -> src16 [16, 128] -----------------
        # src16[p, j] = index of the most recent "moved" timestep <= 128p+j
        # (0 if none): hierarchical cummax of t*move[t].
        src16 = pconst.tile([NT, 128], f32, tag="src16")
        with tc.tile_pool(name="theta_tmp", bufs=4) as prow:
            th_i = prow.tile([NT, 128], i32, tag="th_i")
            nc.sync.dma_start(th_i[:], th_d[:])
            th_f = prow.tile([NT, 128], f32, tag="t16")
            nc.vector.tensor_copy(th_f[:], th_i[:])
            dev = prow.tile([NT, 128], f32, tag="t16")
            nc.vector.tensor_scalar(dev[:], th_f[:], -511.0, None, ALU.add)
            dev2 = prow.tile([NT, 128], f32, tag="t16")
            nc.vector.tensor_tensor(dev2[:], dev[:], dev[:], ALU.mult)
            move = prow.tile([NT, 128], f32, tag="t16")
            nc.vector.tensor_scalar(move[:], dev2[:], 65536.0, None, ALU.is_gt)
            io16 = prow.tile([NT, 128], f32, tag="t16")
            nc.gpsimd.iota(io16[:], pattern=[[1, 128]], base=0,
                           channel_multiplier=128,
                           allow_small_or_imprecise_dtypes=True)
            tmove = prow.tile([NT, 128], f32, tag="t16")
            nc.vector.tensor_tensor(tmove[:], move[:], io16[:], ALU.mult)
            z16 = prow.tile([NT, 128], f32, tag="t16")
            nc.vector.memset(z16[:], 0.0)
            s1 = prow.tile([NT, 128], f32, tag="t16")
            nc.vector.tensor_tensor_scan(s1[:], tmove[:], z16[:], 0.0,
                                         ALU.max, ALU.add)
            # segment maxes live in s1[:, 127]; exclusive cross-segment cummax
            pseg = tc.tile_pool(name="pseg", bufs=1, space="PSUM")
            with pseg as psg:
                sm_t = psg.tile([NT, NT], f32, tag="smt")
                nc.tensor.transpose(sm_t[0:1, 0:NT], s1[:, 127:128],
                                    ident_f[0:NT, 0:NT])
                smrow = prow.tile([1, NT], f32, tag="smrow")
                nc.scalar.activation(smrow[:], sm_t[0:1, 0:NT], AF.Copy)
                incl = prow.tile([1, NT], f32, tag="incl")
                nc.vector.tensor_tensor_scan(incl[:], smrow[:], smrow[:], 0.0,
                                             ALU.max, ALU.bypass)
                excl = prow.tile([1, NT], f32, tag="excl")
                nc.vector.memset(excl[:, 0:1], 0.0)
                nc.vector.tensor_copy(excl[:, 1:NT], incl[:, 0:NT - 1])
                ex_t = psg.tile([NT, NT], f32, tag="smt")
                nc.tensor.transpose(ex_t[0:NT, 0:1], excl[:], ident_f[0:1, 0:1])
                segc = prow.tile([NT, 1], f32, tag="segc")
                nc.scalar.activation(segc[:], ex_t[0:NT, 0:1], AF.Copy)
            nc.vector.tensor_scalar(src16[:], s1[:], segc[:, 0:1], None, ALU.max)
        src_g = pconst.tile([1, T], f32, tag="srcg")
        nc.gpsimd.dma_start(src_g[:], src16[:])

        # ---------------- weights ----------------
        # W_enc split into bf16 hi+lo for the 3-term split GEMM1
        for kc in range(1, KC):
            st = pstage.tile([128, HDIM], f32, tag="westage")
            nc.sync.dma_start(st[:], we_d[128 * kc:128 * (kc + 1), :])
            hi = pconst.tile([128, HDIM], bf16, tag=f"whi{kc}")
            nc.vector.tensor_copy(hi[:], st[:])
            lo = pconst.tile([128, HDIM], bf16, tag=f"wlo{kc}")
            nc.gpsimd.tensor_tensor(lo[:], st[:], hi[:], ALU.subtract)
            whi.append(hi)
            wlo.append(lo)
        wdec_all = pconst.tile([128, HC * IDIM], bf16, tag="wdec")
        for hc in range(HC):
            st = pstage.tile([128, IDIM], f32, tag="wstage")
            nc.sync.dma_start(st[:], wd_d[128 * hc:128 * (hc + 1), :])
            nc.vector.tensor_copy(wdec_all[:, IDIM * hc:IDIM * (hc + 1)], st[:])
        wdec = [wdec_all[:, IDIM * hc:IDIM * (hc + 1)] for hc in range(HC)]

        carry = []
        for j in range(2):
            cro = pconst.tile([1, HDIM], bf16, tag=f"carry{j}", name=f"carry{j}")
            carry.append(cro)

        pbig = ctx.enter_context(tc.tile_pool(name="pbig", bufs=2))
        pew = ctx.enter_context(tc.tile_pool(name="pew", bufs=1))
        pmed = ctx.enter_context(tc.tile_pool(name="pmed", bufs=2))
        psm = ctx.enter_context(tc.tile_pool(name="psm", bufs=2))
        pph = ctx.enter_context(tc.tile_pool(name="pph", bufs=2, space="PSUM"))
        pps = ctx.enter_context(tc.tile_pool(name="pps", bufs=2, space="PSUM"))

        h_bfs, mpress, scrs = {}, {}, {}

        def gemm1_half(i, half, xhi, xlo, h_bf, e_sq, cand, e_work):
            ph = pph.tile([128, 1024], f32, tag="h1024", bufs=2, name="ph")
            # 3-term split: xhi@whi + xhi@wlo + xlo@whi, lhsT-major for LDW reuse
            for kc in range(KC):
                lx = xhi[:, 128 * kc:128 * (kc + 1)]
                for sx in range(2):
                    ns = 1024 * half + 512 * sx
                    nc.tensor.matmul(ph[:, 512 * sx:512 * (sx + 1)],
                                     lhsT=lx, rhs=whi[kc][:, ns:ns + 512],
                                     start=(kc == 0), stop=False)
                for sx in range(2):
                    ns = 1024 * half + 512 * sx
                    nc.tensor.matmul(ph[:, 512 * sx:512 * (sx + 1)],
                                     lhsT=lx, rhs=wlo[kc][:, ns:ns + 512],
                                     start=False, stop=False)
            for kc in range(KC):
                lx = xlo[:, 128 * kc:128 * (kc + 1)]
                for sx in range(2):
                    ns = 1024 * half + 512 * sx
                    nc.tensor.matmul(
                        ph[:, 512 * sx:512 * (sx + 1)],
                        lhsT=lx, rhs=whi[kc][:, ns:ns + 512],
                        start=False,
                        stop=(not use_bias and kc == KC - 1))
            if use_bias:
                for sx in range(2):
                    ns = 1024 * half + 512 * sx
                    nc.tensor.matmul(ph[:, 512 * sx:512 * (sx + 1)],
                                     lhsT=ones_f[:],
                                     rhs=benc[:, ns:ns + 512],
                                     start=False, stop=True)
            nc.scalar.activation(e_sq[:, 1024 * half:1024 * (half + 1)],
                                 ph[:], AF.Square)
            nc.scalar.activation(h_bf[:, 1024 * half:1024 * (half + 1)],
                                 ph[:], AF.Copy)
            for c in range(8 * half, 8 * (half + 1)):
                nc.vector.max(cand[:, 8 * c:8 * (c + 1)],
                              e_sq[:, 128 * c:128 * (c + 1)])
            for c in range(8 * half, 8 * (half + 1)):
                nc.vector.match_replace(e_work[:, 128 * c:128 * (c + 1)],
                                        cand[:, 8 * c:8 * (c + 1)],
                                        e_sq[:, 128 * c:128 * (c + 1)], NEG)

        def front_a(i):
            base = 128 * i
            # x arrives pre-transposed [IDIM, T]: DMA the 4 K-chunks of this
            # tile's columns, split hi/lo on GPSIMD
            if i == 0:
                x_t = x0_t
            else:
                x_t = px.tile([128, 512], f32, tag="x_t", bufs=2, name="x_t")
                for c in range(KC):
                    nc.sync.dma_start(x_t[:, 128 * c:128 * (c + 1)],
                                      x_d[128 * c:128 * (c + 1),
                                          base:base + 128])
            xhi = px.tile([128, 512], bf16, tag="xhi", name="xhi")
            nc.gpsimd.tensor_copy(xhi[:], x_t[:])
            xlo = px.tile([128, 512], bf16, tag="xlo", name="xlo")
            nc.gpsimd.tensor_tensor(xlo[:], x_t[:], xhi[:], ALU.subtract)

            scr = psm.tile([128, 1024], f32, tag="scr", name="scr")
            cand = scr[:, 136:272]
            h_bf = pmed.tile([128, HDIM], bf16, tag="hbf", bufs=3,
                             name="h_bf")
            e_sq = pbig.tile([128, HDIM], f32, tag="e",
                             bufs=3 if not use_bias else 2, name="e_sq")
            h_bfs[i] = h_bf
            e_work = pew.tile([128, HDIM], f32, tag="ew", name="e_work")
            scrs[i] = (scr, xhi, xlo, e_sq, e_work)
            gemm1_half(i, 0, xhi, xlo, h_bf, e_sq, cand, e_work)

        def front_b(i):
            scr, xhi, xlo, e_sq, e_work = scrs[i]
            cand = scr[:, 136:272]
            h_bf = h_bfs[i]
            gemm1_half(i, 1, xhi, xlo, h_bf, e_sq, cand, e_work)

            # exact 64th-largest per row
            nc.vector.max(cand[:, 128:136], e_work[:])
            cur = cand
            c2 = [scr[:, 288:424], scr[:, 432:568]]
            t8s = [scr[:, 576:584], scr[:, 592:600]]
            for r in range(8):
                t8 = t8s[r % 2]
                nc.vector.max(t8, cur[:, 0:136])
                if r < 7:
                    nxt = c2[r % 2]
                    nc.vector.match_replace(nxt, t8, cur[:, 0:136], NEG)
                    cur = nxt
            v64 = scr[:, 608:609]
            nc.vector.tensor_copy(v64, t8[:, 7:8])

            mpres = pmed.tile([128, HDIM], bf16, tag="mp", bufs=3,
                              name="mpres")
            nc.vector.tensor_scalar(mpres[:], e_sq[:], v64[:, 0:1], None,
                                    ALU.is_ge)
            mpress[i] = mpres
            del scrs[i]

        hms = {}

        def back_a(i):
            base = 128 * i
            h_bf = h_bfs.pop(i)
            mpres = mpress.pop(i)
            # permutation matrix for the relay: broadcast src row, compare
            scb = psm.tile([128, 256], bf16, tag="scb", name="scb")
            shifted = psm.tile([128, 8], f32, tag="shifted", name="shifted")
            P_T = scb[:, 0:128]
            cind = scb[0:1, 128:256]
            psb = pps.tile([128, 128], f32, tag="back", bufs=4, name="psb")
            nc.tensor.matmul(psb[:], lhsT=ones_f[:],
                             rhs=src_g[0:1, base:base + 128],
                             start=True, stop=True)
            nc.vector.tensor_scalar(shifted[:, 0:1], io_part_f[:], float(base),
                                    None, ALU.add)
            nc.vector.tensor_scalar(P_T, psb[:], shifted[:, 0:1], None,
                                    ALU.is_equal)
            if i > 0:
                nc.vector.tensor_scalar(cind, src_g[0:1, base:base + 128],
                                        float(base), None, ALU.is_lt)

            mask_sb = pbig.tile([128, HDIM], f32, tag="msk", name="mask_sb")
            hm = pmed.tile([128, HDIM], bf16, tag="hm", name="hm")
            for ns in range(4):
                pm = pps.tile([128, 512], f32, tag="back", bufs=4, name="pm")
                sl = slice(512 * ns, 512 * (ns + 1))
                nc.tensor.matmul(pm[:], lhsT=P_T, rhs=mpres[:, sl],
                                 start=True, stop=(i == 0))
                if i > 0:
                    nc.tensor.matmul(pm[:], lhsT=cind,
                                     rhs=carry[(i - 1) % 2][:, sl],
                                     start=False, stop=True)
                nc.vector.tensor_copy(mask_sb[:, sl], pm[:])
                nc.gpsimd.tensor_tensor(hm[:, sl], h_bf[:, sl], mask_sb[:, sl],
                                        ALU.mult)
            nc.sync.dma_start(mask_d[base:base + 128, :], mask_sb[:])
            if i < NT - 1:
                nc.gpsimd.dma_start(carry[i % 2][:], mask_sb[127:128, :])
            hms[i] = hm

        def back_b(i):
            base = 128 * i
            hm = hms.pop(i)
            hmT = pmed.tile([128, HDIM], bf16, tag="hmT", name="hmT")
            po = pps.tile([128, 512], f32, tag="back", bufs=4, name="po")
            for g in range(4):
                pt = pps.tile([128, 512], bf16, tag="back", bufs=4, name="pt")
                for j in range(4):
                    hc = 4 * g + j
                    nc.tensor.transpose(pt[:, 128 * j:128 * (j + 1)],
                                        hm[:, 128 * hc:128 * (hc + 1)],
                                        ident_b[:])
                nc.scalar.activation(hmT[:, 512 * g:512 * (g + 1)], pt[:],
                                     AF.Copy)
                for j in range(4):
                    hc = 4 * g + j
                    nc.tensor.matmul(po[:],
                                     lhsT=hmT[:, 128 * hc:128 * (hc + 1)],
                                     rhs=wdec[hc][:], start=(hc == 0),
                                     stop=(not use_bias and hc == HC - 1))
            if use_bias:
                nc.tensor.matmul(po[:], lhsT=ones_b[:], rhs=bdec[:],
                                 start=False, stop=True)
            out_sb = px.tile([128, IDIM], f32, tag="out_sb", name="out_sb")
            nc.scalar.activation(out_sb[:], po[:], AF.Copy)
            nc.sync.dma_start(out_d[base:base + 128, :], out_sb[:])

        for i in range(NT + 1):
            if i < NT:
                front_a(i)
                front_b(i)
            if i >= 1:
                back_a(i - 1)
                back_b(i - 1)

    _split_multi_waits(nc, mybir)
    return nc


def run_on_hw(inputs, trace=False, trace_kwargs=None):
    _install_patches()
    from concourse.bass_utils import run_bass_kernel_spmd

    use_bias = bool(np.any(inputs["b_enc"]) or np.any(inputs["b_dec"]))
    key = f"nc_bias{int(use_bias)}"
    if key not in _CACHE:
        _CACHE[key] = build_nc(use_bias=use_bias)
    nc = _CACHE[key]

    x = np.ascontiguousarray(inputs["x"], dtype=np.float32)
    theta = np.ascontiguousarray(inputs["theta"], dtype=np.int32)
    in_maps = []
    for i in range(N_CORES):
        in_maps.append({
            "x": np.ascontiguousarray(x[i].T),
            "theta": theta[i].reshape(NT, 128),
            "W_enc": np.ascontiguousarray(inputs["W_enc"], dtype=np.float32),
            "b_enc": np.ascontiguousarray(inputs["b_enc"], dtype=np.float32)[None, :],
            "W_dec": np.ascontiguousarray(inputs["W_dec"], dtype=np.float32),
            "b_dec": np.ascontiguousarray(inputs["b_dec"], dtype=np.float32)[None, :],
        })
    try:
        res = run_bass_kernel_spmd(nc, in_maps, core_ids=list(range(N_CORES)),
                                   trace=trace, **(trace_kwargs or {}))
    except Exception:
        # transient device/runtime hiccups (e.g. NRT_EXEC_UNIT_UNRECOVERABLE)
        # have been observed to succeed on retry with the same NEFF
        res = run_bass_kernel_spmd(nc, in_maps, core_ids=list(range(N_CORES)),
                                   trace=False)
    out = np.stack([res.results[i]["out"] for i in range(N_CORES)])
    mask = np.stack([res.results[i]["mask"] for i in range(N_CORES)])
    return (out, mask), res


def kernel(x, theta, W_enc, b_enc, W_dec, b_dec):
    (out, mask), _ = run_on_hw(dict(x=x, theta=theta, W_enc=W_enc, b_enc=b_enc,
                                    W_dec=W_dec, b_dec=b_dec))
    return out, mask


if __name__ == "__main__":
    rng = np.random.default_rng(0)
    ins = {
        "x": rng.standard_normal((B, T, IDIM)).astype(np.float32),
        "theta": rng.integers(0, 1024, size=(B, T)).astype(np.int32),
        "W_enc": (rng.standard_normal((IDIM, HDIM)) * 0.02).astype(np.float32),
        "b_enc": np.zeros((HDIM,), np.float32),
        "W_dec": (rng.standard_normal((HDIM, IDIM)) * 0.02).astype(np.float32),
        "b_dec": np.zeros((IDIM,), np.float32),
    }
    (out, mask), res = run_on_hw(ins)
    print("out", out.shape, "mask", mask.shape, "ones/row",
          mask.sum(-1).mean())
